# revision 1
# baseline (speedup 1.0000x reference)
"""Conv2D 3x3 (B=32, C=128, H=W=56 -> OC=256) as a Bass/Tile kernel on 8 NeuronCores.

Strategy: data-parallel over batch (4 images per core), W/b replicated.
The conv is computed as 9 shift-matmuls accumulated in PSUM:
  out[oc, h, w] = sum_{kh,kw} W[oc, :, kh, kw] @ x_pad[:, h+kh, w+kw]
with x zero-padded to 58x58 on the host so every shifted window is a clean
strided view of one SBUF tile. Contraction dim C=128 sits on partitions,
OC=256 is two 128-row output tiles, and the 56x56 output plane is processed
in 7 blocks of 8 rows (N = 8*56 = 448 <= 512, one PSUM bank).

Perf notes (measured on hw):
 - The matmul stream runs at the warm f16 roofline (freedim/2.4GHz spacing,
   LDWEIGHTS hidden by the PE background weight buffer), so the wins are in
   the edges: matmul columns that would only multiply the zero padding are
   trimmed via 3D sub-views of the PSUM tile (center tap goes first with
   start=True so every PSUM element the drain reads is written), and the
   last output-row block is split 6+2 so the final ACT+DMA tail is short.
 - Keep the HAM warm-up burn + single-Sync-queue lead-in: measured, the
   first input DMAs only land ~10.5-12us in (ring spin-up dominates), and
   removing the warm-up makes the first ~13 real matmuls run at the cold
   1.2 GHz clock with DMA stalls resetting the HAM busy window -- a net
   ~2us loss vs burning dummy matmuls while the DMAs fly.

matmul dtype: float16 by default (full PE rate with fast weight load via
FWL; ~3e-4 rel err vs the fp32 reference given this problem's small dynamic
range). Set CONV_MM_DTYPE=f32r (~1.5e-4 err, slower), bf16, or f32 to
switch.
"""

import os

import numpy as np

import concourse.bacc as bacc
import concourse.mybir as mybir
import concourse.tile as tile
from concourse import bass_utils

B, C, H, W_SP = 32, 128, 56, 56
OC, KH, KW = 256, 3, 3
N_CORES = 8
B_PER = B // N_CORES            # 4 images per core
HP, WP = H + 2, W_SP + 2        # zero-padded spatial dims (58x58)
HWP = HP * WP                   # 3364
HWO = H * W_SP                  # 3136
ROWS_PER_TILE = 8               # output rows per matmul tile
N_TILE = ROWS_PER_TILE * W_SP   # 448 (<=512: one PSUM bank)
N_NT = H // ROWS_PER_TILE       # 7
OC_TILES = OC // 128            # 2

# center tap first (always full-region: start=True must cover everything the
# PSUM drain reads), then the kh=1 row, then top/bottom rows -- this is also
# the order the lead-in weight DMAs land in.
TAPS = [(1, 1), (1, 0), (1, 2), (0, 0), (0, 1), (0, 2), (2, 0), (2, 1), (2, 2)]

_NC_CACHE: dict[str, object] = {}


def _mm_mode() -> str:
    return os.environ.get("CONV_MM_DTYPE", "f16")


def _build_nc(mode: str):
    in_dt = {
        "bf16": mybir.dt.bfloat16,
        "f16": mybir.dt.float16,
        "f32r": mybir.dt.float32r,
        "f32": mybir.dt.float32,
    }[mode]
    nc = bacc.Bacc(
        "TRN2",
        target_bir_lowering=False,
        debug=False,
        enable_asserts=False,
        num_devices=N_CORES,
    )
    xp = nc.dram_tensor("xp", [B_PER, C, HWP], in_dt, kind="ExternalInput").ap()
    wt = nc.dram_tensor("wt", [C, KH * KW * OC], in_dt, kind="ExternalInput").ap()
    bias = nc.dram_tensor(
        "bias", [128, OC_TILES], mybir.dt.float32, kind="ExternalInput"
    ).ap()
    # outputs leave the device as f16 (half the DMA bytes; ACT converts on
    # the PSUM drain, the host upcasts back to fp32; adds ~5e-4 rel err)
    out_dt = mybir.dt.float32 if mode.startswith("f32") else mybir.dt.float16
    out = nc.dram_tensor(
        "out", [B_PER, OC, HWO], out_dt, kind="ExternalOutput"
    ).ap()

    CHUNK_ROWS = ROWS_PER_TILE + KH - 1  # 10 padded rows per chunk (2-row halo)

    # Pre-context HAM warm-up: start burning the PE ~1us before the
    # TileContext entry barrier releases (raw bass instructions execute as
    # soon as the PE finishes its framework preamble). The operands are
    # uninitialized garbage -- the results are never read and the PE doesn't
    # trap -- this exists purely to start the clock-gate activity window
    # early. 3 matmuls ~= 1.1us at the cold clock: the PE still reaches the
    # context-entry barrier on time (~7us), so the lead-in DMAs aren't
    # delayed.
    wu_raw = nc.alloc_sbuf_tensor("wu_raw", [C, 512], in_dt)
    psw_raw = nc.alloc_psum_tensor("psw_raw", [128, 512], mybir.dt.float32)
    for i in range(2):
        nc.tensor.matmul(
            psw_raw.ap()[:, :320],
            wu_raw.ap()[:, :128],
            wu_raw.ap()[:, :320],
            start=(i == 0),
            stop=(i == 1),
        )

    with tile.TileContext(nc) as tc:
        with (
            tc.tile_pool(name="xin", bufs=16) as xpool,
            tc.tile_pool(name="wpool", bufs=1) as wpool,
            tc.tile_pool(name="bpool", bufs=1) as bpool,
            tc.tile_pool(name="opool", bufs=4) as opool,
            tc.tile_pool(name="psum", bufs=4, space="PSUM") as pspool,
        ):
            # In-context HAM warm-up continues the pre-context burn: dummy
            # matmuls until the first weight/chunk DMAs land (~10.5us), so
            # the real stream starts at the warm 2.4 GHz clock.
            wu = wpool.tile([C, 512], in_dt, tag="wu")
            nc.gpsimd.memset(wu[:], 0.0)
            psw = pspool.tile([128, 512], mybir.dt.float32, tag="ps")
            for i in range(7):
                nc.tensor.matmul(
                    psw[:, :N_TILE],
                    wu[:, :128],
                    wu[:, :N_TILE],
                    start=(i == 0),
                    stop=(i == 6),
                )

            # lead-in DMAs, finest first in matmul need-order (the k-th
            # matmul of the first PSUM group needs tap TAPS[k] and chunk 0).
            # Bias rides GpSimd (off the critical Sync issue queue).
            wsb = wpool.tile([C, KH * KW, OC], in_dt, tag="wsb")
            wtv = wt.rearrange("c (k m) -> c k m", m=OC)
            xv0 = xp[0].rearrange("c (h w) -> c h w", w=WP)
            nc.sync.dma_start(wsb[:, 4, :], wtv[:, 4, :])  # (1,1) center
            xc0 = xpool.tile([C, CHUNK_ROWS, WP], in_dt, tag="xc")
            nc.sync.dma_start(xc0[:], xv0[:, :CHUNK_ROWS, :])
            nc.sync.dma_start(wsb[:, 3, :], wtv[:, 3, :])  # (1,0)
            nc.sync.dma_start(wsb[:, 5, :], wtv[:, 5, :])  # (1,2)
            nc.sync.dma_start(wsb[:, 0:3, :], wtv[:, 0:3, :])  # kh=0 row
            nc.sync.dma_start(wsb[:, 6:9, :], wtv[:, 6:9, :])  # kh=2 row
            bsb = bpool.tile([128, OC_TILES], mybir.dt.float32, tag="bsb")
            nc.gpsimd.dma_start(bsb[:], bias[:])

            for img in range(B_PER):
                xv = xp[img].rearrange("c (h w) -> c h w", w=WP)
                for nt in range(N_NT):
                    blk0 = nt * ROWS_PER_TILE
                    if img == 0 and nt == 0:
                        xc = xc0
                    else:
                        xc = xpool.tile([C, CHUNK_ROWS, WP], in_dt, tag="xc")
                        nc.sync.dma_start(
                            xc[:], xv[:, blk0 : blk0 + CHUNK_ROWS, :]
                        )
                    for oc_t in range(OC_TILES):
                        # the very last group is split 6+2 so the final
                        # ACT+DMA drain after the last matmul is short
                        is_last = (
                            img == B_PER - 1
                            and nt == N_NT - 1
                            and oc_t == OC_TILES - 1
                        )
                        subs = [(0, 6), (6, 2)] if is_last else [(0, ROWS_PER_TILE)]
                        for sr, nr in subs:
                            n_free = nr * W_SP
                            ps = pspool.tile(
                                [128, ROWS_PER_TILE, W_SP],
                                mybir.dt.float32,
                                tag="ps",
                            )
                            n_taps = len(TAPS)
                            for ki, (kh, kw) in enumerate(TAPS):
                                # trim rows/cols whose input is all zero
                                # padding; the center tap (ki==0) is always
                                # full so start=True covers the drain region
                                r0, r1 = sr, sr + nr
                                if nt == 0 and kh == 0:
                                    r0 = max(r0, 1)
                                if nt == N_NT - 1 and kh == 2:
                                    r1 = min(r1, ROWS_PER_TILE - 1)
                                c0, c1 = 0, W_SP
                                if kw == 0:
                                    c0 = 1
                                elif kw == 2:
                                    c1 = W_SP - 1
                                rhs = xc[:, kh + r0 : kh + r1, kw + c0 : kw + c1]
                                lhsT = wsb[
                                    :,
                                    kh * KW + kw,
                                    oc_t * 128 : (oc_t + 1) * 128,
                                ]
                                nc.tensor.matmul(
                                    ps[:, r0 - sr : r1 - sr, c0:c1],
                                    lhsT,
                                    rhs,
                                    start=(ki == 0),
                                    stop=(ki == n_taps - 1),
                                )
                            ot = opool.tile(
                                [128, N_TILE], out_dt, tag="ot"
                            )
                            psf = ps[:, :nr, :].rearrange("p r c -> p (r c)")
                            nc.scalar.activation(
                                ot[:, :n_free],
                                psf,
                                mybir.ActivationFunctionType.Identity,
                                bias=bsb[:, oc_t : oc_t + 1],
                            )
                            col0 = nt * N_TILE + sr * W_SP
                            # out-DMAs alternate between the Sync and ACT
                            # hw-DGE rings: halves the per-ring issue load and
                            # keeps the ACT ring warm. The two last-group subs
                            # split across the rings so the final sub's DMA
                            # issue isn't queued behind the 6-row sub's ~0.6us
                            # issue on the same ring.
                            if is_last:
                                dma_eng = nc.scalar if sr > 0 else nc.sync
                            else:
                                dma_eng = nc.scalar if oc_t == 1 else nc.sync
                            dma_eng.dma_start(
                                out[
                                    img,
                                    oc_t * 128 : (oc_t + 1) * 128,
                                    col0 : col0 + n_free,
                                ],
                                ot[:, :n_free],
                            )
    nc.compile()
    return nc


def _get_nc(mode: str):
    nc = _NC_CACHE.get(mode)
    if nc is None:
        nc = _build_nc(mode)
        _NC_CACHE[mode] = nc
    return nc


def kernel(x: np.ndarray, W: np.ndarray, b: np.ndarray) -> np.ndarray:
    mode = _mm_mode()
    x = np.asarray(x, dtype=np.float32)
    W = np.asarray(W, dtype=np.float32)
    b = np.asarray(b, dtype=np.float32)

    if mode == "bf16":
        import ml_dtypes

        in_np_dt = ml_dtypes.bfloat16
    elif mode == "f16":
        in_np_dt = np.float16
    else:
        in_np_dt = np.float32

    # Host-side layout prep: zero-pad x spatially, put the conv taps of W
    # into [tap, C, OC] (lhsT layout), stripe bias to [128, OC_TILES].
    xp = np.zeros((B, C, HP, WP), dtype=in_np_dt)
    xp[:, :, 1:-1, 1:-1] = x
    xp = xp.reshape(N_CORES, B_PER, C, HWP)
    # wt[c, k*OC + oc] = W[oc, c*9 + k]  (lhsT tap blocks, contiguous per c)
    wt = np.ascontiguousarray(
        W.reshape(OC, C, KH * KW).transpose(1, 2, 0).reshape(C, KH * KW * OC)
    ).astype(in_np_dt)
    bias = np.ascontiguousarray(b.reshape(OC_TILES, 128).T).astype(np.float32)

    nc = _get_nc(mode)
    in_maps = [
        {"xp": np.ascontiguousarray(xp[i]), "wt": wt, "bias": bias}
        for i in range(N_CORES)
    ]
    trace = os.environ.get("CONV_TRACE", "") not in ("", "0")
    try:
        res = bass_utils.run_bass_kernel_spmd(
            nc,
            in_maps,
            core_ids=list(range(N_CORES)),
            trace=trace,
        )
    except Exception:
        # transient device wedges (NRT_EXEC_UNIT_UNRECOVERABLE) have been
        # observed once; a fresh dispatch usually recovers
        import time

        time.sleep(2.0)
        res = bass_utils.run_bass_kernel_spmd(
            nc,
            in_maps,
            core_ids=list(range(N_CORES)),
            trace=trace,
        )
    kernel._last_results = res  # for test harness introspection
    out = np.stack([res.results[i]["out"] for i in range(N_CORES)])
    return out.reshape(B, OC, H, W_SP).astype(np.float32)



# revision 2
# speedup vs baseline: 1.0359x; 1.0359x over previous
"""Conv2D 3x3 (B=32, C=128, H=W=56 -> OC=256) via 1D Winograd F(2,3) on 8 cores.

Data-parallel over batch (4 images/core). Width dim uses Winograd F(2,3):
output col pair (2t, 2t+1) needs 4 transformed inputs x~[j] = B^T d over
padded cols 2t..2t+3; height dim stays direct (3 kh shift-taps accumulated
in PSUM). Per 14-row output block and oc-tile: 12 matmuls (4 j-planes x 3
kh) of free dim 392 (14 rows x 28 tile-cols) -> 2/3 the PE columns of the
direct 9-tap conv (62us vs 92us PE floor per core).

The input transform (4 shifted +/- of x columns) is done on the HOST --
it is linear O(N) data prep like the zero-padding, and doubles the input
DMA bytes (6.6MB/core, still well under the PE time on the Sync ring)
while freeing the on-chip vector engines entirely for the output side.

Output transform a = z0+z1+z2+bias (even cols), b = z1-z2-z3+bias (odd),
split so each PSUM plane is read exactly once (~87G elem/s per engine):
  ACT:    s1 = Id(z1 + bias) -> SBUF;  s2 = Id(z2) -> SBUF
  DVE:    a' = z0 + s1;  b' = (-z3) + s1;  a = a' + s2 -> f16 even cols
  GpSimd: b = b' - s2 -> f16 odd cols (SBUF-only operands)
Out-DMAs alternate Sync/ACT hw-DGE rings.

Weights are Winograd-transformed on the host per kh: w~[j] in
{w0, (w0+w1+w2)/2, (w0-w1+w2)/2, w2}, laid out [C, 12, OC] f16 in matmul
need-order (j emission order 1,0,2,3; kh order 1,0,2 so the start=True
matmul always covers the full plane and edge-row trims ride start=False).
"""

import os

import numpy as np

import concourse.bacc as bacc
import concourse.mybir as mybir
import concourse.tile as tile
from concourse import bass_utils

B, C, H, W_SP = 32, 128, 56, 56
OC, KH, KW = 256, 3, 3
N_CORES = 8
B_PER = B // N_CORES            # 4 images per core
HP = H + 2                      # padded rows (58)
N_J = 4                         # winograd positions per tile
TW = W_SP // 2                  # 28 tiles across width
ROWS = 14                       # output rows per block
N_BLK = H // ROWS               # 4
CHUNK_ROWS = ROWS + KH - 1      # 16 padded rows per chunk
NFREE = ROWS * TW               # 392
OC_TILES = OC // 128            # 2
HWO = H * W_SP                  # 3136
XT_FREE = N_J * HP * TW         # per-channel x~ elements per image (6496)

# tap emission order: j in (1,2,3,0), kh in (1,0,2). kh=1 first => the
# start=True matmul covers full rows. j-order staggers the PSUM plane
# stops so every drain pass except a'/a starts BEFORE the group's last
# matmul: s1(z1)@mm3, s2(z2)@mm6, b'(z3)@mm9; only a'(z0)+a trail the
# group (~1.2us on DVE < the 2-buf PSUM rotation tolerance).
J_ORDER = (1, 2, 3, 0)
KH_ORDER = (1, 0, 2)
N_WARMUP = int(os.environ.get("CONV_WARMUP", "6"))
N_TAILBURN = int(os.environ.get("CONV_TAILBURN", "28"))

_NC_CACHE: dict[str, object] = {}


def _build_nc(mode: str = "f16"):
    in_dt = mybir.dt.float16
    f32 = mybir.dt.float32
    nc = bacc.Bacc(
        "TRN2",
        target_bir_lowering=False,
        debug=False,
        enable_asserts=False,
        num_devices=N_CORES,
    )
    # host-transformed input: [img, C, j, padded_row, tile_col]
    xt_d = nc.dram_tensor(
        "xt", [B_PER, C, XT_FREE], in_dt, kind="ExternalInput"
    ).ap()
    wt = nc.dram_tensor("wt", [C, 12 * OC], in_dt, kind="ExternalInput").ap()
    bias = nc.dram_tensor(
        "bias", [128, 2 * OC_TILES], f32, kind="ExternalInput"
    ).ap()
    out = nc.dram_tensor("out", [B_PER, OC, HWO], f32, kind="ExternalOutput").ap()

    with tile.TileContext(nc) as tc:
        with (
            tc.tile_pool(name="xt", bufs=8) as xtpool,
            tc.tile_pool(name="wpool", bufs=1) as wpool,
            tc.tile_pool(name="bpool", bufs=1) as bpool,
            tc.tile_pool(name="s1p", bufs=8) as s1pool,
            tc.tile_pool(name="ap", bufs=4) as apool,
            tc.tile_pool(name="bp", bufs=4) as bppool,
            tc.tile_pool(name="opool", bufs=6) as opool,
            tc.tile_pool(name="ps0", bufs=2, space="PSUM") as pspool0,
            tc.tile_pool(name="ps1", bufs=2, space="PSUM") as pspool1,
            tc.tile_pool(name="ps2", bufs=2, space="PSUM") as pspool2,
            tc.tile_pool(name="ps3", bufs=2, space="PSUM") as pspool3,
        ):
            pspools = [pspool0, pspool1, pspool2, pspool3]
            # HAM warm-up: burn the PE while the lead-in DMAs land so the
            # real stream starts at the warm clock.
            wu = wpool.tile([C, 512], in_dt, tag="wu")
            nc.gpsimd.memset(wu[:], 0.0)
            psw = pspool0.tile([128, 512], f32, tag="z0")
            for i in range(N_WARMUP):
                nc.tensor.matmul(
                    psw[:, :],
                    wu[:, :128],
                    wu[:, :512],
                    start=(i == 0),
                    stop=(i == N_WARMUP - 1),
                )

            # lead-in DMAs: first input chunk (longest dep chain), the 3
            # j=1 weight taps, second chunk, remaining taps. Bias rides
            # GpSimd's software DGE.
            wsb = wpool.tile([C, 12, OC], in_dt, tag="wsb")
            wtv = wt.rearrange("c (k m) -> c k m", m=OC)
            bsb = bpool.tile([128, 2 * OC_TILES], f32, tag="bsb")

            chunks = [(img, blk) for img in range(B_PER) for blk in range(N_BLK)]

            def chunk_dma(ci, split=False):
                img, blk = chunks[ci]
                xv = xt_d[img].rearrange("c (j h w) -> c j h w", j=N_J, w=TW)
                xc = xtpool.tile([C, N_J, CHUNK_ROWS, TW], in_dt, tag="xc")
                r0 = blk * ROWS
                # input chunks ride the ACT ring (its queue is just s1/s2)
                # so they never queue behind output transfers on Sync
                if split:
                    # lead-in: land planes in matmul need order (J_ORDER)
                    for j in J_ORDER:
                        nc.scalar.dma_start(
                            xc[:, j], xv[:, j, r0 : r0 + CHUNK_ROWS, :]
                        )
                else:
                    nc.scalar.dma_start(xc[:], xv[:, :, r0 : r0 + CHUNK_ROWS, :])
                return xc

            xc_bufs = {}
            xc_bufs[0] = chunk_dma(0, split=True)
            nc.sync.dma_start(wsb[:, 0:3, :], wtv[:, 0:3, :])
            xc_bufs[1] = chunk_dma(1)
            nc.sync.dma_start(wsb[:, 3:12, :], wtv[:, 3:12, :])
            xc_bufs[2] = chunk_dma(2)
            xc_bufs[3] = chunk_dma(3)
            nc.gpsimd.dma_start(bsb[:], bias[:])


            add = mybir.AluOpType.add
            mult = mybir.AluOpType.mult

            gi = 0  # group counter for DMA-ring alternation
            pending_dmas = []  # (dst, src, engine) deferred 2 groups so the
            # dma_start never head-of-line blocks s1/s1d (ACT) or chunk
            # prefetch (Sync) behind an unfinished ot tile
            for ci, (img, blk) in enumerate(chunks):
                if ci + 4 < len(chunks):
                    xc_bufs[ci + 4] = chunk_dma(ci + 4)
                xc = xc_bufs.pop(ci)

                is_last_chunk = ci == len(chunks) - 1
                for oc_t in range(OC_TILES):
                    subs = (
                        [(0, 10), (10, 4)]
                        if (is_last_chunk and oc_t == OC_TILES - 1)
                        else [(0, ROWS)]
                    )
                    # per-plane PSUM pools: a rewrite of plane j only waits
                    # plane j's reader (tile-level WAR tracking would gate
                    # the whole group on the last drain pass + ~600ns
                    # sem-post latency per cross-engine hop)
                    for sr, nr in subs:
                        nfree = nr * TW
                        psj = {}
                        for j in J_ORDER:
                            ztile = pspools[j].tile([128, 512], f32, tag=f"z{j}")
                            psj[j] = ztile
                        for jj, j in enumerate(J_ORDER):
                            for ki, kh in enumerate(KH_ORDER):
                                r0 = 1 if (blk == 0 and kh == 0) else sr
                                r1 = (
                                    sr + nr - 1
                                    if (blk == N_BLK - 1 and kh == 2 and sr + nr == ROWS)
                                    else sr + nr
                                )
                                rhs = xc[:, j, kh + r0 : kh + r1, :]
                                lhsT = wsb[
                                    :, jj * 3 + ki, oc_t * 128 : (oc_t + 1) * 128
                                ]
                                nc.tensor.matmul(
                                    psj[j][:, (r0 - sr) * TW : (r1 - sr) * TW],
                                    lhsT,
                                    rhs,
                                    start=(ki == 0),
                                    stop=(ki == len(KH_ORDER) - 1),
                                )

                        z = [psj[j][:, :nfree] for j in range(4)]
                        # Shallow 2-hop drain graph (deep cross-engine chains
                        # cascade into a latency-bound pipeline): ACT drains
                        # z1 and z2 independently; DVE's three STT passes and
                        # GpSimd's one SBUF-only TT hang off s1/s2.
                        s1 = s1pool.tile([128, NFREE], f32, tag="s1")
                        nc.scalar.activation(
                            s1[:, :nfree],
                            z[1],
                            mybir.ActivationFunctionType.Identity,
                            bias=bsb[:, oc_t : oc_t + 1],
                        )
                        s2 = s1pool.tile([128, NFREE], f32, tag="s2")
                        nc.scalar.activation(
                            s2[:, :nfree],
                            z[2],
                            mybir.ActivationFunctionType.Identity,
                        )
                        ap_ = apool.tile([128, NFREE], f32, tag="ap")
                        bp_ = bppool.tile([128, NFREE], f32, tag="bp")
                        # out keeps a/b planes separate (host interleaves the
                        # even/odd cols) so every vector pass is contiguous;
                        # fp32 out (TENSOR_TENSOR and f16-out passes run at
                        # half rate, ~1.05us vs 0.57us for fp32 STT).
                        ot = opool.tile([128, 2, NFREE], f32, tag="ot")
                        # a = z0+s1+z2 -> plane 0; b = s1-z2-z3 -> plane 1
                        nc.vector.scalar_tensor_tensor(
                            bp_[:, :nfree], z[3], -1.0, s1[:, :nfree], mult, add
                        )
                        nc.gpsimd.tensor_sub(
                            ot[:, 1, :nfree], bp_[:, :nfree], s2[:, :nfree]
                        )
                        nc.vector.scalar_tensor_tensor(
                            ap_[:, :nfree], z[0], 0.0, s1[:, :nfree], add, add
                        )
                        # DVE's 3rd STT pass saturates it (102%); the final
                        # SBUF-only combine alternates onto GpSimd
                        if gi % 2:
                            nc.gpsimd.tensor_add(
                                ot[:, 0, :nfree], ap_[:, :nfree], s2[:, :nfree]
                            )
                        else:
                            nc.vector.scalar_tensor_tensor(
                                ot[:, 0, :nfree], ap_[:, :nfree], 0.0,
                                s2[:, :nfree], add, add,
                            )

                        ov = out[img].rearrange(
                            "o (e h w) -> o e h w", e=2, w=TW
                        )
                        if is_last_chunk:
                            dma_eng = nc.scalar if gi % 2 else nc.sync
                        else:
                            dma_eng = nc.gpsimd if gi % 2 else nc.sync
                        pending_dmas.append(
                            (
                                ov[
                                    oc_t * 128 : (oc_t + 1) * 128,
                                    :,
                                    blk * ROWS + sr : blk * ROWS + sr + nr,
                                    :,
                                ],
                                ot[:, :, :nfree],
                                dma_eng,
                            )
                        )
                        keep = 0 if is_last_chunk else 2
                        while len(pending_dmas) > keep:
                            dst, src, eng = pending_dmas.pop(0)
                            eng.dma_start(dst, src)
                        gi += 1
            # tail burn: dummy matmuls keep the core clock (DVFS) up while
            # the last groups drain + their DMAs fly -- with the PE idle the
            # vector drains run ~3x slower (2.4us stt vs 0.57us warm)
            psb = pspool1.tile([128, 512], f32, tag="z1")
            for i in range(N_TAILBURN):
                nc.tensor.matmul(
                    psb[:, :],
                    wu[:, :128],
                    wu[:, :512],
                    start=(i == 0),
                    stop=(i == N_TAILBURN - 1),
                )
            # final flush rides the HW rings (ACT ring is free of chunk
            # DMAs by now; GpSimd swdge transfers are ~2x slower)
            for fi, (dst, src, eng) in enumerate(pending_dmas):
                (nc.scalar if fi % 2 else nc.sync).dma_start(dst, src)
    nc.compile()
    return nc


def _get_nc(mode: str):
    nc = _NC_CACHE.get(mode)
    if nc is None:
        nc = _build_nc(mode)
        _NC_CACHE[mode] = nc
    return nc


def kernel(x: np.ndarray, W: np.ndarray, b: np.ndarray) -> np.ndarray:
    x = np.asarray(x, dtype=np.float32)
    W = np.asarray(W, dtype=np.float32)
    b = np.asarray(b, dtype=np.float32)
    in_np_dt = np.float16

    # zero-pad x spatially, then host-side 1D Winograd input transform
    # along width: d_k = padded col 2t+k; x~ planes j0..j3.
    xp = np.zeros((B, C, HP, H + 2), dtype=np.float32)
    xp[:, :, 1:-1, 1:-1] = x
    d0 = xp[:, :, :, 0:56:2]
    d1 = xp[:, :, :, 1:57:2]
    d2 = xp[:, :, :, 2:58:2]
    d3 = xp[:, :, :, 3:58:2]
    xt = np.empty((B, C, N_J, HP, TW), dtype=in_np_dt)
    xt[:, :, 0] = d0 - d2
    xt[:, :, 1] = d1 + d2
    xt[:, :, 2] = d2 - d1
    xt[:, :, 3] = d1 - d3
    xt = xt.reshape(N_CORES, B_PER, C, XT_FREE)

    # Winograd-transform W along kw per kh: wt[c, pos, oc] with pos in
    # matmul need-order (j in 1,0,2,3) x (kh in 1,0,2)
    wf = W.reshape(OC, C, KH, KW)
    w0, w1, w2 = wf[:, :, :, 0], wf[:, :, :, 1], wf[:, :, :, 2]
    wj = {
        0: w0,
        1: (w0 + w1 + w2) * 0.5,
        2: (w0 - w1 + w2) * 0.5,
        3: w2,
    }  # each [OC, C, KH]
    taps = []
    for j in J_ORDER:
        for kh in KH_ORDER:
            taps.append(wj[j][:, :, kh].T)  # [C, OC]
    wt = np.ascontiguousarray(
        np.stack(taps, axis=1).reshape(C, 12 * OC)
    ).astype(in_np_dt)
    b_cols = b.reshape(OC_TILES, 128).T
    bias = np.ascontiguousarray(
        np.concatenate([b_cols, 2.0 * b_cols], axis=1)
    ).astype(np.float32)

    nc = _get_nc("f16")
    in_maps = [
        {"xt": np.ascontiguousarray(xt[i]), "wt": wt, "bias": bias}
        for i in range(N_CORES)
    ]
    trace = os.environ.get("CONV_TRACE", "") not in ("", "0")
    try:
        res = bass_utils.run_bass_kernel_spmd(
            nc, in_maps, core_ids=list(range(N_CORES)), trace=trace
        )
    except Exception:
        import time

        time.sleep(2.0)
        res = bass_utils.run_bass_kernel_spmd(
            nc, in_maps, core_ids=list(range(N_CORES)), trace=trace
        )
    kernel._last_results = res
    out = np.stack([res.results[i]["out"] for i in range(N_CORES)])
    # device emits [img, oc, {a,b}, h, tw]; interleave a/b into even/odd cols
    out = out.reshape(B, OC, 2, H, TW).astype(np.float32)
    full = np.empty((B, OC, H, W_SP), dtype=np.float32)
    full[:, :, :, 0::2] = out[:, :, 0]
    full[:, :, :, 1::2] = out[:, :, 1]
    return full
